# revision 34
# baseline (speedup 1.0000x reference)
"""TRN2 Bass kernel for nn_DFT: out = log((x @ Wr.T)^2 + (x @ Wi.T)^2).

x: [262144, 256] f32;  dft_real/dft_imag: [256, 256] f32 (symmetric DFT mats).

Strategy
--------
Data-parallel over 8 NeuronCores: each core handles 32768 rows (frames).

Math: x is real, so the spectrum is conjugate-symmetric: mag[b, k] ==
mag[b, 256-k]. The device computes k = 0..127; the host mirrors k =
129..255 and computes the two real-only columns k = 0 and k = 128
exactly (they are plain +-1-weighted sums - 0.8% of the FLOPs - and are
the chi^2_1 cancellation-prone columns where fp16-split precision on
the device would not track the fp32 reference).

Device math (mode "q16", the default): plain fp16 matmuls
r ~= fp16(x) @ fp16(W) with fp32 PSUM accumulation. The resulting
~2e-2 absolute error in r/i only harms log(r^2+i^2) where the
magnitude is small, so the host recomputes every entry whose device
log is below FIX_T exactly (~30k of 33.5M entries, found from the
device's own output - iterative-refinement style). This makes the
device stream 4 matmuls per 512 columns instead of the 12 a 3-term
fp16 hi/lo split needs (mode "h16", kept as the higher-precision
fallback: 185 us, device absmax ~8e-3 vs f64).

Layout: device works in transposed (frequency-major) orientation.
Host packs xT per core as [128, NPAIR*2*1024] fp16 so each pair of
512-col groups needs ONE input DMA (the Sync queue serializes DMA
issues at ~700 ns each). Output is fp16 [128, 32768] (halves out-DMA
bytes; log range +-24 -> abs err ~1e-2 vs the 0.47 absmax budget).

Measured: fp32 mode 289 us -> h16 185 us -> q16 108 us. See
_build_q16's docstring for the engine balance and rejected variants.
"""

import numpy as np

NFFT = 256
BATCH = 262144
N_CORES = 8
B_CORE = BATCH // N_CORES  # 32768
NB = 512                   # moving-dim tile (matmul max, one PSUM bank)
NG = B_CORE // NB          # 64 groups

MODE = "q16"

NP = 1024                  # q16: paired-group tile (2 PSUM banks)
NPAIR = B_CORE // NP       # 32 pairs
FIX_T = -1.5               # host recomputes entries with device log < FIX_T

_PROG_CACHE = {}


def bass_ts(i, size):
    return slice(i * size, (i + 1) * size)


def _build_program(mode):
    import concourse.bacc as bacc
    import concourse.mybir as mybir
    import concourse.tile as tile

    nc = bacc.Bacc("TRN2", target_bir_lowering=False, debug=False)
    if mode == "q16":
        return _build_q16(nc, mybir, tile)
    if mode == "h16":
        return _build_h16(nc, mybir, tile)
    if mode == "fp32":
        return _build_fp32(nc, mybir, tile)
    raise ValueError(mode)


def _build_q16(nc, mybir, tile):
    """Single-term fp16 matmuls: r ~= fp16(x) @ fp16(W). The ~2e-2 absolute
    matmul error only matters where |X|^2 is small; the host recomputes all
    entries whose device log is < FIX_T exactly (~30k of 33.5M). 4 matmuls
    per 512-col group = 1/3 of a 3-term hi/lo split's PE time, and half its
    input bytes. Groups are processed in pairs sharing [128, 1024] PSUM
    tiles so each ScalarE/VectorE instruction covers 1024 columns (the
    ~350-cycle ACT fixed overhead would otherwise dominate at this pace).

    Engine balance per pair (~2.6 us): PE 8 data + 4 identity matmuls
    (2.6), ACT square+Ln (2.3), DVE cast+square (2.4), Sync 2 DMA issues,
    DMA 768 KB. Measured variants: DVE-add instead of identity matmuls ->
    VectorE-bound (128 us); GpSimd CCE-DMA add -> SWDGE-bound (172 us);
    ACT/DVE column-split squares -> overlap-inefficient (134 us); quad-
    batched DMAs + deeper ps_r -> chip power-throttle, every engine -20%
    (127 us). This arrangement with deep prefetch (xpool bufs=6): 108 us;
    shallow prefetch (bufs=3) stalls the PE on input-DMA jitter (124 us).
    """
    f16 = mybir.dt.float16
    f32 = mybir.dt.float32
    Ln = mybir.ActivationFunctionType.Ln
    A = mybir.AluOpType

    xin = nc.dram_tensor("xin", [128, NPAIR * 2 * NP], f16, kind="ExternalInput").ap()
    wpk = nc.dram_tensor("wpk", [NFFT, NFFT], f16, kind="ExternalInput").ap()
    idn = nc.dram_tensor("idn", [128, 128], f16, kind="ExternalInput").ap()
    outT = nc.dram_tensor("outT", [128, B_CORE], f16, kind="ExternalOutput").ap()

    with tile.TileContext(nc) as tc:
        with (
            tc.tile_pool(name="wpool", bufs=1) as wpool,
            tc.tile_pool(name="xpool", bufs=8) as xpool,
            tc.tile_pool(name="pspool", bufs=2, space="PSUM") as pspool,
            tc.tile_pool(name="sqpool", bufs=5) as sqpool,
            tc.tile_pool(name="opool", bufs=5) as opool,
        ):
            # HAM warmup from a memset tile (no DMA dependency).
            wrm = wpool.tile([128, NB], f16, tag="wrm")
            nc.gpsimd.memset(wrm[:], 1.0)
            ps_wu = pspool.tile([128, NP], f32, tag="ps_r")
            for _ in range(8):
                nc.tensor.matmul(
                    ps_wu[:, 0:NB], wrm[:, 0:128], wrm[:],
                    start=True, stop=True, skip_group_check=True,
                )

            # First pair's x, split so the first matmuls' data lands ASAP.
            xt0 = xpool.tile([128, 2 * NP], f16, tag="xt")
            nc.sync.dma_start(xt0[:, 0:NP], xin[:, 0:NP])

            # Weights resident: wpk rows j, cols 0:128 w_re, 128:256 w_im.
            wt0 = wpool.tile([128, NFFT], f16, tag="wt0")
            nc.sync.dma_start(wt0[:], wpk[0:128, :])
            wt1 = wpool.tile([128, NFFT], f16, tag="wt1")
            nc.sync.dma_start(wt1[:], wpk[128:256, :])
            idt = wpool.tile([128, 128], f16, tag="idt")
            nc.sync.dma_start(idt[:], idn)

            nc.sync.dma_start(xt0[:, NP : 2 * NP], xin[:, NP : 2 * NP])

            def stage_a(xt):
                """Data matmuls + squares for one pair."""
                ps_r = pspool.tile([128, NP], f32, tag="ps_r")
                ps_i = pspool.tile([128, NP], f32, tag="ps_i")
                for half in range(NP // NB):  # 512-col matmul slices
                    c0 = half * NB
                    ps0 = slice(c0, c0 + NB)
                    xs0 = slice(0 * NP + c0, 0 * NP + c0 + NB)  # j 0:128
                    xs1 = slice(1 * NP + c0, 1 * NP + c0 + NB)  # j 128:256
                    nc.tensor.matmul(ps_r[:, ps0], wt0[:, 0:128],
                                     xt[:, xs0], start=True, stop=False)
                    nc.tensor.matmul(ps_r[:, ps0], wt1[:, 0:128],
                                     xt[:, xs1], start=False, stop=True)
                    nc.tensor.matmul(ps_i[:, ps0], wt0[:, 128:256],
                                     xt[:, xs0], start=True, stop=False)
                    nc.tensor.matmul(ps_i[:, ps0], wt1[:, 128:256],
                                     xt[:, xs1], start=False, stop=True)
                sq_r = sqpool.tile([128, NP], f16, tag="sq_r")
                nc.scalar.square(sq_r[:], ps_r[:])
                ci = sqpool.tile([128, NP], f16, tag="ci")
                nc.vector.tensor_copy(ci[:], ps_i[:])
                sq_i = sqpool.tile([128, NP], f16, tag="sq_i")
                nc.vector.scalar_tensor_tensor(
                    sq_i[:], ci[:], 1.0, ci[:], op0=A.mult, op1=A.mult,
                )
                return ps_r, sq_r, sq_i

            def stage_b(p, ps_r, sq_r, sq_i):
                """m = r^2 + i^2 on the PE (two accumulating identity
                matmuls overwrite ps_r, already consumed by the square),
                then Ln straight from PSUM, then the output DMA. ScalarE
                and VectorE are near their occupancy limits at this pace;
                the PE is not."""
                o16 = opool.tile([128, NP], f16, tag="o16")
                last = p == NPAIR - 1
                for half in range(NP // NB):
                    ps0 = slice(half * NB, (half + 1) * NB)
                    nc.tensor.matmul(ps_r[:, ps0], idt[:], sq_r[:, ps0],
                                     start=True, stop=False)
                    nc.tensor.matmul(ps_r[:, ps0], idt[:], sq_i[:, ps0],
                                     start=False, stop=True)
                    if last:
                        # halve the final chain: Ln+store of the first half
                        # overlap the second half's sum matmuls
                        nc.scalar.activation(o16[:, ps0], ps_r[:, ps0], Ln)
                        nc.sync.dma_start(
                            outT[:, p * NP + ps0.start : p * NP + ps0.stop],
                            o16[:, ps0])
                if not last:
                    nc.scalar.activation(o16[:], ps_r[:], Ln)
                    nc.sync.dma_start(outT[:, p * NP : (p + 1) * NP], o16[:])

            # Software-pipelined by one pair: pair p's sum/Ln/store are
            # emitted after pair p+1's matmuls+squares. ScalarE's queue is
            # strict FIFO, so emitting [... sq(p), Ln(p), sq(p+1) ...] would
            # head-of-line-block sq(p+1) behind Ln(p)'s semaphore wait and
            # stall the PE; the pipelined order [... sq(p), sq(p+1), Ln(p),
            # ...] keeps every engine's next instruction runnable.
            pend = None
            for p in range(NPAIR):
                if p == 0:
                    xt = xt0
                else:
                    xt = xpool.tile([128, 2 * NP], f16, tag="xt")
                    nc.sync.dma_start(xt[:], xin[:, p * 2 * NP : (p + 1) * 2 * NP])
                cur = stage_a(xt)
                if pend is not None:
                    stage_b(*pend)
                pend = (p, *cur)
            stage_b(*pend)

    nc.compile()
    return nc


def _build_h16(nc, mybir, tile):
    f16 = mybir.dt.float16
    f32 = mybir.dt.float32
    Ln = mybir.ActivationFunctionType.Ln
    A = mybir.AluOpType

    xin = nc.dram_tensor("xin", [128, NG * 4 * NB], f16, kind="ExternalInput").ap()
    wpk = nc.dram_tensor("wpk", [NFFT, 512], f16, kind="ExternalInput").ap()
    outT = nc.dram_tensor("outT", [128, B_CORE], f16, kind="ExternalOutput").ap()

    with tile.TileContext(nc) as tc:
        with (
            tc.tile_pool(name="wpool", bufs=1) as wpool,
            tc.tile_pool(name="xpool", bufs=4) as xpool,
            tc.tile_pool(name="pspool", bufs=2, space="PSUM") as pspool,
            tc.tile_pool(name="sqpool", bufs=4) as sqpool,
            tc.tile_pool(name="opool", bufs=4) as opool,
        ):
            # Warmup operand that depends on no DMA: a memset tile. The
            # dummy matmuls trip the PE HAM activity window during the
            # framework boot + first-DMA latency, so the real stream starts
            # at 2.4 GHz instead of ramping from 1.2 GHz ~3.4 us in.
            wrm = wpool.tile([128, NB], f16, tag="wrm")
            nc.vector.memset(wrm[:], 1.0)
            ps_w = pspool.tile([128, NB], f32, tag="ps_w")
            for _ in range(6):
                nc.tensor.matmul(
                    ps_w[:], wrm[:, 0:128], wrm[:],
                    start=True, stop=True, skip_group_check=True,
                )

            # First x group, split so the first matmuls' data lands ASAP.
            xt0 = xpool.tile([128, 4 * NB], f16, tag="xt")
            nc.sync.dma_start(xt0[:, 0 : 2 * NB], xin[:, 0 : 2 * NB])

            # Weights resident for the whole kernel.
            # wpk rows j (contraction), cols: 0:128 wh_re, 128:256 wh_im,
            # 256:384 wl_re, 384:512 wl_im.
            wt0 = wpool.tile([128, 512], f16, tag="wt0")
            nc.sync.dma_start(wt0[:], wpk[0:128, :])
            wt1 = wpool.tile([128, 512], f16, tag="wt1")
            nc.sync.dma_start(wt1[:], wpk[128:256, :])

            nc.sync.dma_start(xt0[:, 2 * NB : 4 * NB], xin[:, 2 * NB : 4 * NB])

            for g in range(NG):
                cs = bass_ts(g, NB)
                if g == 0:
                    xt = xt0
                else:
                    xt = xpool.tile([128, 4 * NB], f16, tag="xt")
                    nc.sync.dma_start(xt[:], xin[:, g * 4 * NB : (g + 1) * 4 * NB])

                # Last group: process in column halves so the elementwise +
                # output-DMA chain of the first half overlaps the second
                # half's matmuls, shortening the kernel tail.
                nh = 2 if g == NG - 1 else 1
                H = NB // nh
                for h in range(nh):
                    hs = slice(h * H, (h + 1) * H)
                    ocs = slice(g * NB + h * H, g * NB + (h + 1) * H)
                    xh0 = xt[:, 0 * NB + h * H : 0 * NB + (h + 1) * H]
                    xh1 = xt[:, 1 * NB + h * H : 1 * NB + (h + 1) * H]
                    xl0 = xt[:, 2 * NB + h * H : 2 * NB + (h + 1) * H]
                    xl1 = xt[:, 3 * NB + h * H : 3 * NB + (h + 1) * H]

                    ps_r = pspool.tile([128, H], f32, tag="ps_r")
                    nc.tensor.matmul(ps_r[:], wt0[:, 0:128], xh0, start=True, stop=False)
                    nc.tensor.matmul(ps_r[:], wt1[:, 0:128], xh1, start=False, stop=False)
                    nc.tensor.matmul(ps_r[:], wt0[:, 256:384], xh0, start=False, stop=False)
                    nc.tensor.matmul(ps_r[:], wt1[:, 256:384], xh1, start=False, stop=False)
                    nc.tensor.matmul(ps_r[:], wt0[:, 0:128], xl0, start=False, stop=False)
                    nc.tensor.matmul(ps_r[:], wt1[:, 0:128], xl1, start=False, stop=True)

                    ps_i = pspool.tile([128, H], f32, tag="ps_i")
                    nc.tensor.matmul(ps_i[:], wt0[:, 128:256], xh0, start=True, stop=False)
                    nc.tensor.matmul(ps_i[:], wt1[:, 128:256], xh1, start=False, stop=False)
                    nc.tensor.matmul(ps_i[:], wt0[:, 384:512], xh0, start=False, stop=False)
                    nc.tensor.matmul(ps_i[:], wt1[:, 384:512], xh1, start=False, stop=False)
                    nc.tensor.matmul(ps_i[:], wt0[:, 128:256], xl0, start=False, stop=False)
                    nc.tensor.matmul(ps_i[:], wt1[:, 128:256], xl1, start=False, stop=True)

                    sq_r = sqpool.tile([128, H], f32, tag="sq_r")
                    nc.scalar.square(sq_r[:], ps_r[:])
                    sq_i = sqpool.tile([128, H], f32, tag="sq_i")
                    nc.scalar.square(sq_i[:], ps_i[:])
                    sq_f = sqpool.tile([128, H], f32, tag="sq_f")
                    nc.vector.scalar_tensor_tensor(
                        sq_f[:], sq_r[:], 1.0, sq_i[:], op0=A.mult, op1=A.add,
                    )
                    o16 = opool.tile([128, H], f16, tag="o16")
                    nc.scalar.activation(o16[:], sq_f[:], Ln)
                    nc.sync.dma_start(outT[:, ocs], o16[:])

    nc.compile()
    return nc


def _build_fp32(nc, mybir, tile):
    """Proven fallback: fp32 matmuls, 289 us measured."""
    f32 = mybir.dt.float32
    Ln = mybir.ActivationFunctionType.Ln
    NOUT = NFFT // 2 + 1

    xT = nc.dram_tensor("xT", [NFFT, B_CORE], f32, kind="ExternalInput").ap()
    w = nc.dram_tensor("w", [NFFT, NFFT], f32, kind="ExternalInput").ap()
    outT = nc.dram_tensor("outT", [NOUT, B_CORE], f32, kind="ExternalOutput").ap()

    with tile.TileContext(nc) as tc:
        with (
            tc.tile_pool(name="wpool", bufs=1) as wpool,
            tc.tile_pool(name="xpool", bufs=4) as xpool,
            tc.tile_pool(name="pspool", bufs=4, space="PSUM") as pspool,
            tc.tile_pool(name="sqpool", bufs=4) as sqpool,
            tc.tile_pool(name="opool", bufs=4) as opool,
            tc.tile_pool(name="lpool", bufs=4) as lpool,
        ):
            wt0 = wpool.tile([128, NFFT], f32, tag="wt0")
            nc.sync.dma_start(wt0[:], w[0:128, :])
            wt1 = wpool.tile([128, NFFT], f32, tag="wt1")
            nc.sync.dma_start(wt1[:], w[128:256, :])
            mask = wpool.tile([128, 1], f32, tag="mask")
            nc.vector.memset(mask[:], 1.0)
            nc.vector.memset(mask[0:1, :], 0.0)

            for g in range(NG):
                cs = bass_ts(g, NB)
                x0 = xpool.tile([128, NB], f32, tag="x0")
                nc.sync.dma_start(x0[:], xT[0:128, cs])
                x1 = xpool.tile([128, NB], f32, tag="x1")
                nc.sync.dma_start(x1[:], xT[128:256, cs])

                ps_r = pspool.tile([128, NB], f32, tag="ps_r")
                nc.tensor.matmul(ps_r[:], wt0[:, 0:128], x0[:], start=True, stop=False)
                nc.tensor.matmul(ps_r[:], wt1[:, 0:128], x1[:], start=False, stop=True)
                ps_i = pspool.tile([128, NB], f32, tag="ps_i")
                nc.tensor.matmul(ps_i[:], wt0[:, 128:256], x0[:], start=True, stop=False)
                nc.tensor.matmul(ps_i[:], wt1[:, 128:256], x1[:], start=False, stop=True)

                sq_r = sqpool.tile([128, NB], f32, tag="sq_r")
                nc.scalar.square(sq_r[:], ps_r[:])
                sq_i = sqpool.tile([128, NB], f32, tag="sq_i")
                nc.scalar.square(sq_i[:], ps_i[:])

                o_last = lpool.tile([1, NB], f32, tag="o_last")
                nc.scalar.activation(o_last[:], sq_i[0:1, :], Ln)

                sq_f = sqpool.tile([128, NB], f32, tag="sq_f")
                nc.vector.scalar_tensor_tensor(
                    sq_f[:], sq_i[:], mask[:], sq_r[:],
                    op0=mybir.AluOpType.mult, op1=mybir.AluOpType.add,
                )

                o_main = opool.tile([128, NB], f32, tag="o_main")
                nc.scalar.activation(o_main[:], sq_f[:], Ln)

                nc.sync.dma_start(outT[0:128, cs], o_main[:])
                nc.sync.dma_start(outT[128:129, cs], o_last[:])

    nc.compile()
    return nc


def _get_program(mode):
    if mode not in _PROG_CACHE:
        _PROG_CACHE[mode] = _build_program(mode)
    return _PROG_CACHE[mode]


def _make_wfull(dft_real, dft_imag):
    # [256 contraction, 256 outputs]: cols 0:128 real k=0..127,
    # 128:256 imag k=0..127 (imag col 0 is naturally all-zero).
    return np.concatenate(
        [dft_real[0:128, :].T, dft_imag[0:128, :].T], axis=1
    ).astype(np.float32)


def _prep_core_q16(xc):
    """xc [B_CORE, 256] f32 -> fp16 packed [128, NPAIR*2*NP]."""
    xh = xc.astype(np.float16)
    # [B, 256] -> [NPAIR, NP, 2, 128] -> [128, NPAIR, 2, NP]
    a = xh.reshape(NPAIR, NP, 2, 128).transpose(3, 0, 2, 1)
    return np.ascontiguousarray(a.reshape(128, NPAIR * 2 * NP))


def _prep_core_h16(xc):
    """xc [B_CORE, 256] f32 -> packed [128, NG*4*NB] fp16."""
    xh = xc.astype(np.float16)
    xl = (xc - xh.astype(np.float32)).astype(np.float16)
    # [B, 256] -> [NG, NB, 2, 128] -> [128, NG, 2, NB]
    a = xh.reshape(NG, NB, 2, 128).transpose(3, 0, 2, 1)
    b = xl.reshape(NG, NB, 2, 128).transpose(3, 0, 2, 1)
    xin = np.concatenate([a, b], axis=2)  # [128, NG, 4, NB]
    return np.ascontiguousarray(xin.reshape(128, NG * 4 * NB))


def _run(x, dft_real, dft_imag, trace=False, tmpdir=None):
    import concourse.bass_utils as bass_utils

    nc = _get_program(MODE)
    full = np.empty((BATCH, NFFT), dtype=np.float32)

    if MODE == "q16":
        wfull = _make_wfull(dft_real, dft_imag)
        wpk = np.ascontiguousarray(wfull.astype(np.float16))
        idn = np.eye(128, dtype=np.float16)
        in_maps = []
        for c in range(N_CORES):
            xc = x[c * B_CORE : (c + 1) * B_CORE, :]
            in_maps.append({"xin": _prep_core_q16(xc), "wpk": wpk, "idn": idn})
        res = bass_utils.run_bass_kernel_spmd(
            nc, in_maps, core_ids=list(range(N_CORES)), trace=trace, tmpdir=tmpdir
        )
        for c in range(N_CORES):
            block = res.results[c]["outT"]  # [128, B_CORE] f16
            full[c * B_CORE : (c + 1) * B_CORE, 0:128] = block.T.astype(np.float32)
        # Exact real-only columns (DC and Nyquist): chi^2_1 cancellation
        # makes them precision-critical; they are +-1-weighted sums.
        x64 = x.astype(np.float64)
        s0 = x64.sum(axis=1)
        s128 = x64[:, ::2].sum(axis=1) - x64[:, 1::2].sum(axis=1)
        full[:, 0] = np.log(s0 * s0)
        full[:, 128] = np.log(s128 * s128)
        # Precision fixup: the fp16 matmul's ~2e-2 absolute error only harms
        # log where |X|^2 is small. Recompute those entries exactly (~30k of
        # 33.5M; `~(> T)` also catches NaN/-inf from fp16 underflow).
        dev = full[:, 1:128]
        bs, ks = np.nonzero(~(dev > FIX_T))
        ks = ks + 1
        xr = x64[bs, :]
        r = np.einsum("ij,ij->i", xr, dft_real[ks, :].astype(np.float64))
        im = np.einsum("ij,ij->i", xr, dft_imag[ks, :].astype(np.float64))
        full[bs, ks] = np.log(r * r + im * im)
    elif MODE == "h16":
        wfull = _make_wfull(dft_real, dft_imag)
        wh = wfull.astype(np.float16)
        wl = (wfull - wh.astype(np.float32)).astype(np.float16)
        wpk = np.ascontiguousarray(np.concatenate([wh, wl], axis=1))
        in_maps = []
        for c in range(N_CORES):
            xc = x[c * B_CORE : (c + 1) * B_CORE, :]
            in_maps.append({"xin": _prep_core_h16(xc), "wpk": wpk})
        res = bass_utils.run_bass_kernel_spmd(
            nc, in_maps, core_ids=list(range(N_CORES)), trace=trace, tmpdir=tmpdir
        )
        for c in range(N_CORES):
            block = res.results[c]["outT"]  # [128, B_CORE] f16
            full[c * B_CORE : (c + 1) * B_CORE, 0:128] = block.T.astype(np.float32)
        # Exact real-only columns (DC and Nyquist): chi^2_1 cancellation
        # makes them precision-critical; they are +-1-weighted sums.
        x64 = x.astype(np.float64)
        s0 = x64.sum(axis=1)
        s128 = x64[:, ::2].sum(axis=1) - x64[:, 1::2].sum(axis=1)
        full[:, 0] = np.log(s0 * s0)
        full[:, 128] = np.log(s128 * s128)
    else:  # fp32 fallback
        NOUT = NFFT // 2 + 1
        wr_half = dft_real[0:128, :]
        wi_half = dft_imag[0:128, :].copy()
        wi_half[0, :] = dft_real[128, :]
        wfull = np.ascontiguousarray(
            np.concatenate([wr_half.T, wi_half.T], axis=1).astype(np.float32)
        )
        in_maps = []
        for c in range(N_CORES):
            xc = x[c * B_CORE : (c + 1) * B_CORE, :]
            in_maps.append({"xT": np.ascontiguousarray(xc.T), "w": wfull})
        res = bass_utils.run_bass_kernel_spmd(
            nc, in_maps, core_ids=list(range(N_CORES)), trace=trace, tmpdir=tmpdir
        )
        for c in range(N_CORES):
            block = res.results[c]["outT"]  # [129, B_CORE]
            full[c * B_CORE : (c + 1) * B_CORE, 0:NOUT] = block.T

    full[:, NFFT // 2 + 1 :] = full[:, NFFT // 2 - 1 : 0 : -1]
    return full, res


def kernel(x, dft_real, dft_imag):
    x = np.asarray(x, dtype=np.float32)
    dft_real = np.asarray(dft_real, dtype=np.float32)
    dft_imag = np.asarray(dft_imag, dtype=np.float32)
    full, _ = _run(x, dft_real, dft_imag, trace=False)
    return full
